# revision 12
# baseline (speedup 1.0000x reference)
"""AttentionPairBias kernel for 8 Trainium2 NeuronCores.

Sharding: data-parallel over (batch, query-row-block). Core c handles batch
b = c // 4 and query rows i in [(c % 4) * 128, (c % 4 + 1) * 128).
Each core computes the full 16-head attention for its 128 query rows:
  - q/g projections for its rows; k/v projections for its batch (replicated
    across the 4 cores of the batch).
  - pair bias via the LayerNorm decomposition
      bias[i,j,h] = rsig(i,j) * (zu[i,j,h] - mu(i,j) * su[h]) + t[h]
    with u[:,h] = ln_g * wz[:,h], su = sum_c u, t = ln_b @ wz, so the only
    full-z work is one matmul zT.T @ [u | ones] (done in bf16 on the PE with
    4-way column tiling) plus a squared copy for the variance.
  - z arrives host-transposed as zT [c_z, i, j] so the contraction dim is on
    partitions; it is cast fp32->bf16 during the DMA load.
  - zu / musum / sumsq round-trip through DRAM to switch from [head, (i,j)]
    layout back to [i, j] tiles.
"""

import sys

sys.path.insert(0, "/opt/trn_rl_repo")

from contextlib import ExitStack

import numpy as np

import concourse.bacc as bacc
import concourse.bass as bass
import concourse.mybir as mybir
import concourse.tile as tile
from concourse.bass_utils import run_bass_kernel_spmd
from concourse.masks import make_identity

F32 = mybir.dt.float32
F32R = mybir.dt.float32r
BF16 = mybir.dt.bfloat16
AF = mybir.ActivationFunctionType
ALU = mybir.AluOpType

B, N, CS, CZ, H, D = 2, 512, 1024, 128, 16, 64
ROWS = 128          # query rows per core
NCHUNK = CS // 128  # 8 contraction chunks of 128
N_CORES = 8
EPS = 1e-5

_CACHE = {}


def _build_program(mask_trivial: bool):
    nc = bacc.Bacc("TRN2", target_bir_lowering=False, debug=False,
                   num_devices=N_CORES)

    def din(name, shape):
        return nc.dram_tensor(name, shape, F32, kind="ExternalInput").ap()

    sT_d = din("sT", (CS, ROWS))
    kinT_d = din("kinT", (CS, N))
    # bf16 hi/lo planes of zT, bit-packed into f32-typed tensors (the axon
    # PJRT compile path stalls on non-f32 jit parameters).
    zh_d = din("zTh", (CZ, ROWS, N // 2))
    zl_d = din("zTl", (CZ, ROWS, N // 2))
    wq_d = din("wq", (CS, CS))
    wk_d = din("wk", (CS, CS))
    wv_d = din("wv", (CS, CS))
    wg_d = din("wg", (CS, CS))
    wo_d = din("wo", (CS, CS))
    bq_d = din("bqt", (128, NCHUNK))
    lng_d = din("lng", (CZ, 1))
    lnb_d = din("lnb", (CZ, 1))
    wz_d = din("wz", (CZ, H))
    if not mask_trivial:
        mneg_d = din("mneg", (1, N))
    out_d = nc.dram_tensor("out", (ROWS, CS), F32, kind="ExternalOutput").ap()

    with tile.TileContext(nc) as tc, ExitStack() as ctx:
        dram = ctx.enter_context(tc.tile_pool(name="dram", bufs=1, space="DRAM"))
        zu_d = dram.tile([17, ROWS, N], F32)     # [head|musum, i, j]
        ss_d = dram.tile([ROWS, N], F32)         # sumsq over c per (i, j)

        const = ctx.enter_context(tc.tile_pool(name="const", bufs=1))
        small = ctx.enter_context(tc.tile_pool(name="small", bufs=1))

        ident = const.tile([128, 128], F32)
        make_identity(nc, ident[:])
        ones = const.tile([128, 128], F32)
        nc.vector.memset(ones[:], 1.0)

        wz_sb = small.tile([CZ, H], F32)
        nc.sync.dma_start(wz_sb[:], wz_d[:])
        lng_sb = small.tile([CZ, 1], F32)
        nc.sync.dma_start(lng_sb[:], lng_d[:])
        lnb_sb = small.tile([CZ, 1], F32)
        nc.sync.dma_start(lnb_sb[:], lnb_d[:])
        bq_sb = small.tile([128, NCHUNK], F32)
        nc.sync.dma_start(bq_sb[:], bq_d[:])

        u_f = small.tile([CZ, H], F32)
        nc.vector.tensor_tensor(u_f[:], wz_sb[:],
                                lng_sb[:, 0:1].to_broadcast([CZ, H]), ALU.mult)
        bwz = small.tile([CZ, H], F32)
        nc.vector.tensor_tensor(bwz[:], wz_sb[:],
                                lnb_sb[:, 0:1].to_broadcast([CZ, H]), ALU.mult)
        # stationaries for the z matmul, hi/lo split of u:
        #   u1 = [u_hi (16) | ones | zeros...], u2 = [u_lo (16) | zeros...]
        u_bf = const.tile([CZ, 32], BF16)
        nc.vector.memset(u_bf[:], 0.0)
        nc.vector.tensor_copy(u_bf[:, 0:H], u_f[:])
        nc.vector.memset(u_bf[:, H:H + 1], 1.0)
        u_hi_f = small.tile([CZ, H], F32)
        nc.vector.tensor_copy(u_hi_f[:], u_bf[:, 0:H])
        u_lo = const.tile([CZ, 32], BF16)
        nc.vector.memset(u_lo[:], 0.0)
        u_lo_f = small.tile([CZ, H], F32)
        nc.vector.tensor_tensor(u_lo_f[:], u_f[:], u_hi_f[:], ALU.subtract)
        nc.vector.tensor_copy(u_lo[:, 0:H], u_lo_f[:])

        msu_b = small.tile([128, H], F32)   # -su[h]/128 replicated on partitions
        t_b = small.tile([128, H], F32)
        with ExitStack() as pctx:
            ppre = pctx.enter_context(tc.tile_pool(name="ppre", bufs=1,
                                                   space="PSUM"))
            su_ps = ppre.tile([128, H], F32, tag="pre")
            nc.tensor.matmul(su_ps[:], ones[:], u_f[:], start=True, stop=True)
            nc.vector.tensor_scalar_mul(msu_b[:], su_ps[:], -1.0 / CZ)
            t_ps = ppre.tile([128, H], F32, tag="pre")
            nc.tensor.matmul(t_ps[:], ones[:], bwz[:], start=True, stop=True)
            nc.vector.tensor_copy(t_b[:], t_ps[:])
        bq8 = small.tile([128, NCHUNK], F32)
        nc.vector.tensor_scalar_mul(bq8[:], bq_sb[:], 0.125)

        if not mask_trivial:
            mrow = small.tile([1, N], F32)
            nc.sync.dma_start(mrow[:], mneg_d[:])
            mfull = small.tile([128, N], F32)
            nc.vector.tensor_copy(mfull[:], mrow[0:1, :].to_broadcast([128, N]))

        # ---------------- phase 1: z -> zu / musum / sumsq ----------------
        QR = 4   # query rows per (group, step)
        with ExitStack() as zctx:
            ztp = zctx.enter_context(tc.tile_pool(name="ztp", bufs=8))
            z2p = zctx.enter_context(tc.tile_pool(name="z2p", bufs=8))
            zup = zctx.enter_context(tc.tile_pool(name="zup", bufs=2))
            ssp = zctx.enter_context(tc.tile_pool(name="ssp", bufs=2))
            zps = zctx.enter_context(tc.tile_pool(name="zps", bufs=3, space="PSUM"))

            for o in range(32 // QR):
                zts, zls, z2s = [], [], []
                for g in range(4):
                    r0 = 32 * g + QR * o
                    zt32 = ztp.tile([CZ, QR, N // 2], F32, tag="zt")
                    nc.sync.dma_start(zt32[:], zh_d[:, r0:r0 + QR, :])
                    zl32 = ztp.tile([CZ, QR, N // 2], F32, tag="zl")
                    nc.sync.dma_start(zl32[:], zl_d[:, r0:r0 + QR, :])
                    zt = zt32[:].bitcast(BF16)
                    zl = zl32[:].bitcast(BF16)
                    z2 = z2p.tile([CZ, QR, N], BF16, tag="z2")
                    nc.scalar.activation(z2[:], zt[:], AF.Square)
                    zts.append(zt)
                    zls.append(zl)
                    z2s.append(z2)
                zu_sb = zup.tile([128, QR, N], F32)
                ss_sb = ssp.tile([128, QR, N], F32)
                for kk in range(QR):
                    ps = zps.tile([128, 2 * N], F32)
                    for g in range(4):
                        tp = (0, 32 * g)
                        dst = ps[32 * g:32 * g + 32, 0:N]
                        nc.tensor.matmul(dst, u_bf[:], zts[g][:, kk, :],
                                         start=True, stop=False,
                                         tile_position=tp)
                        nc.tensor.matmul(dst, u_lo[:], zts[g][:, kk, :],
                                         start=False, stop=False,
                                         tile_position=tp)
                        nc.tensor.matmul(dst, u_bf[:], zls[g][:, kk, :],
                                         start=False, stop=True,
                                         tile_position=tp)
                        nc.tensor.matmul(ps[32 * g:32 * g + 32, N:2 * N],
                                         u_bf[:], z2s[g][:, kk, :],
                                         start=True, stop=True,
                                         tile_position=tp)
                    nc.vector.tensor_copy(zu_sb[:, kk, :], ps[:, 0:N])
                    nc.scalar.copy(ss_sb[:, kk, :], ps[:, N:2 * N])
                for g in range(4):
                    r0 = 32 * g + QR * o
                    nc.sync.dma_start(zu_d[:, r0:r0 + QR, :],
                                      zu_sb[32 * g:32 * g + 17, :, :])
                    nc.sync.dma_start(
                        ss_d[r0:r0 + QR, :].rearrange("(o k) j -> o k j", o=1),
                        ss_sb[32 * g + 16:32 * g + 17, :, :])

        # ---------------- phase 2: projections ----------------
        proj = ctx.enter_context(tc.tile_pool(name="proj", bufs=1))
        sT_sb = proj.tile([128, NCHUNK, ROWS], F32)
        nc.sync.dma_start(sT_sb[:], sT_d.rearrange("(cc p) i -> p cc i", p=128))
        kinT_sb = proj.tile([128, NCHUNK, N], F32)
        nc.sync.dma_start(kinT_sb[:], kinT_d.rearrange("(cc p) j -> p cc j", p=128))

        qT_sb = proj.tile([128, NCHUNK, ROWS], F32)    # (q + bq)/8, [d, i]
        kT_sb = proj.tile([128, NCHUNK, N], F32)       # [d, j]
        v_sb = proj.tile([128, 4, CS], F32)            # [j within chunk, jc, h*64+d]
        g_sb = proj.tile([128, CS], F32)               # sigmoid(s @ wg), [i, c]

        with ExitStack() as wctx:
            wpool = wctx.enter_context(tc.tile_pool(name="wpool", bufs=2))
            prps = wctx.enter_context(tc.tile_pool(name="prps", bufs=2, space="PSUM"))

            wq_sb = wpool.tile([128, NCHUNK, CS], F32, tag="w")
            nc.sync.dma_start(wq_sb[:], wq_d.rearrange("(cc p) n -> p cc n", p=128))
            for dc in range(NCHUNK):
                ps = prps.tile([128, ROWS], F32, tag="q")
                for cc in range(NCHUNK):
                    nc.tensor.matmul(ps[:], wq_sb[:, cc, 128 * dc:128 * dc + 128],
                                     sT_sb[:, cc, :],
                                     start=(cc == 0), stop=(cc == NCHUNK - 1))
                nc.scalar.activation(qT_sb[:, dc, :], ps[:], AF.Identity,
                                     bias=bq8[:, dc:dc + 1], scale=0.125)

            wk_sb = wpool.tile([128, NCHUNK, CS], F32, tag="w")
            nc.sync.dma_start(wk_sb[:], wk_d.rearrange("(cc p) n -> p cc n", p=128))
            for dc in range(NCHUNK):
                ps = prps.tile([128, N], F32, tag="k")
                for cc in range(NCHUNK):
                    nc.tensor.matmul(ps[:],
                                     wk_sb[:, cc, 128 * dc:128 * dc + 128],
                                     kinT_sb[:, cc, :],
                                     start=(cc == 0), stop=(cc == NCHUNK - 1))
                nc.scalar.copy(kT_sb[:, dc, :], ps[:])

            wv_sb = wpool.tile([128, NCHUNK, CS], F32, tag="w")
            nc.sync.dma_start(wv_sb[:], wv_d.rearrange("(cc p) n -> p cc n", p=128))
            for jc in range(4):
                for nh in range(2):
                    ps = prps.tile([128, 512], F32, tag="v")
                    for cc in range(NCHUNK):
                        nc.tensor.matmul(
                            ps[:],
                            kinT_sb[:, cc, 128 * jc:128 * jc + 128],
                            wv_sb[:, cc, 512 * nh:512 * nh + 512],
                            start=(cc == 0), stop=(cc == NCHUNK - 1))
                    nc.scalar.copy(v_sb[:, jc, 512 * nh:512 * nh + 512], ps[:])

            wg_sb = wpool.tile([128, NCHUNK, CS], F32, tag="w")
            nc.sync.dma_start(wg_sb[:], wg_d.rearrange("(cc p) n -> p cc n", p=128))
            for nh in range(2):
                ps = prps.tile([128, 512], F32, tag="v")
                for cc in range(NCHUNK):
                    nc.tensor.matmul(ps[:], sT_sb[:, cc, :],
                                     wg_sb[:, cc, 512 * nh:512 * nh + 512],
                                     start=(cc == 0), stop=(cc == NCHUNK - 1))
                nc.scalar.activation(g_sb[:, 512 * nh:512 * nh + 512], ps[:],
                                     AF.Sigmoid)

        # ---------------- phase 3: attention ----------------
        att = ctx.enter_context(tc.tile_pool(name="att", bufs=3))
        apool = ctx.enter_context(tc.tile_pool(name="apool", bufs=1))
        spsum = ctx.enter_context(tc.tile_pool(name="spsum", bufs=2, space="PSUM"))
        tpsum = ctx.enter_context(tc.tile_pool(name="tpsum", bufs=2, space="PSUM"))
        opsum = ctx.enter_context(tc.tile_pool(name="opsum", bufs=2, space="PSUM"))

        musum = apool.tile([128, N], F32)
        nc.sync.dma_start(musum[:],
                          zu_d[16:17, :, :].rearrange("o i j -> (o i) j"))
        ssq = apool.tile([128, N], F32)
        nc.sync.dma_start(ssq[:], ss_d[:])
        m2 = apool.tile([128, N], F32)
        nc.vector.tensor_tensor(m2[:], musum[:], musum[:], ALU.mult)
        wvar = apool.tile([128, N], F32)   # 128 * var
        nc.vector.scalar_tensor_tensor(wvar[:], m2[:], -1.0 / CZ, ssq[:],
                                       op0=ALU.mult, op1=ALU.add)
        eps_b = apool.tile([128, 1], F32)
        nc.vector.memset(eps_b[:], EPS)
        sdev = apool.tile([128, N], F32)   # sqrt(var + eps)
        nc.scalar.activation(sdev[:], wvar[:], AF.Sqrt, bias=eps_b[:, 0:1],
                             scale=1.0 / CZ)
        rsig = apool.tile([128, N], F32)
        nc.vector.reciprocal(rsig[:], sdev[:])

        wo_sb = proj.tile([128, NCHUNK, CS], F32)
        nc.sync.dma_start(wo_sb[:], wo_d.rearrange("(cc p) n -> p cc n", p=128))

        o_all = apool.tile([128, H, D], F32)
        sums = apool.tile([128, H], F32)

        for h in range(H):
            bh = att.tile([128, N], F32, tag="bh")
            nc.vector.tensor_scalar_mul(bh[:], musum[:], msu_b[:, h:h + 1])
            if not mask_trivial:
                nc.vector.tensor_tensor(bh[:], bh[:], mfull[:], ALU.add)
            nc.gpsimd.dma_start(
                bh[:], zu_d[h:h + 1, :, :].rearrange("o i j -> (o i) j"),
                accum_op=ALU.add)
            sc_ps = spsum.tile([128, N], F32, tag="sc")
            p0 = 64 * (h % 2)
            nc.tensor.matmul(sc_ps[:],
                             qT_sb[p0:p0 + 64, h // 2, :],
                             kT_sb[p0:p0 + 64, h // 2, :],
                             start=True, stop=True)
            t2 = att.tile([128, N], F32, tag="t2")
            nc.vector.tensor_tensor(t2[:], bh[:], rsig[:], ALU.mult)
            s_sb = att.tile([128, N], F32, tag="s")
            nc.vector.scalar_tensor_tensor(s_sb[:], t2[:], t_b[:, h:h + 1],
                                           sc_ps[:], op0=ALU.add, op1=ALU.add)
            nm = att.tile([128, 1], F32, tag="nm")
            nc.vector.tensor_reduce(nm[:], s_sb[:], mybir.AxisListType.X,
                                    ALU.max, negate=True)
            p_sb = att.tile([128, N], F32, tag="p")
            nc.scalar.activation(p_sb[:], s_sb[:], AF.Exp, bias=nm[:, 0:1],
                                 accum_out=sums[:, h:h + 1])
            pt_ps = tpsum.tile([128, N], F32, tag="pt")
            for jc in range(4):
                nc.tensor.transpose(pt_ps[:, 128 * jc:128 * jc + 128],
                                    p_sb[:, 128 * jc:128 * jc + 128], ident[:])
            pt_sb = att.tile([128, N], F32, tag="ptsb")
            nc.vector.tensor_copy(pt_sb[:], pt_ps[:])
            o_ps = opsum.tile([128, D], F32, tag="o")
            for jc in range(4):
                nc.tensor.matmul(o_ps[:], pt_sb[:, 128 * jc:128 * jc + 128],
                                 v_sb[:, jc, D * h:D * h + D],
                                 start=(jc == 0), stop=(jc == 3))
            nc.scalar.copy(o_all[:, h, :], o_ps[:])

        recip = apool.tile([128, H], F32)
        nc.vector.reciprocal(recip[:], sums[:])
        go = apool.tile([128, H, D], F32)
        nc.vector.tensor_tensor(go[:], o_all[:],
                                recip[:, :, None].to_broadcast([128, H, D]),
                                ALU.mult)
        gof = go.rearrange("p h d -> p (h d)")
        nc.vector.tensor_tensor(gof[:], gof[:], g_sb[:], ALU.mult)

        goT = apool.tile([128, NCHUNK, ROWS], F32)
        for ccc in range(NCHUNK):
            gt_ps = tpsum.tile([128, 128], F32, tag="pt")
            nc.tensor.transpose(gt_ps[:], gof[:, 128 * ccc:128 * ccc + 128],
                                ident[:])
            nc.scalar.copy(goT[:, ccc, :], gt_ps[:])

        out_sb = apool.tile([128, CS], F32)
        for nh in range(2):
            ps = spsum.tile([128, 512], F32, tag="sc")
            for cc in range(NCHUNK):
                nc.tensor.matmul(ps[:], goT[:, cc, :],
                                 wo_sb[:, cc, 512 * nh:512 * nh + 512],
                                 start=(cc == 0), stop=(cc == NCHUNK - 1))
            nc.vector.tensor_copy(out_sb[:, 512 * nh:512 * nh + 512], ps[:])
        nc.sync.dma_start(out_d[:], out_sb[:])

    nc.compile()
    return nc


def _prepare(s, z, mask, k_in, wq, bq, wk, wv, wg, ln_g, ln_b, wz, wo,
             multiplicity=1, **_ignored):
    import ml_dtypes
    s = np.asarray(s, dtype=np.float32)
    z = np.asarray(z, dtype=np.float32)
    mask = np.asarray(mask, dtype=np.float32)
    k_in = np.asarray(k_in, dtype=np.float32)
    assert int(multiplicity) == 1, "only multiplicity == 1 is supported"
    mask_trivial = bool(np.all(mask == 1.0))

    shared = {
        "wq": np.ascontiguousarray(wq, dtype=np.float32),
        "wk": np.ascontiguousarray(wk, dtype=np.float32),
        "wv": np.ascontiguousarray(wv, dtype=np.float32),
        "wg": np.ascontiguousarray(wg, dtype=np.float32),
        "wo": np.ascontiguousarray(wo, dtype=np.float32),
        "bqt": np.ascontiguousarray(
            np.asarray(bq, dtype=np.float32).reshape(NCHUNK, 128).T),
        "lng": np.ascontiguousarray(
            np.asarray(ln_g, dtype=np.float32).reshape(CZ, 1)),
        "lnb": np.ascontiguousarray(
            np.asarray(ln_b, dtype=np.float32).reshape(CZ, 1)),
        "wz": np.ascontiguousarray(wz, dtype=np.float32),
    }
    in_maps = []
    for core in range(N_CORES):
        b, ib = core // 4, core % 4
        i0 = ib * ROWS
        m = dict(shared)
        m["sT"] = np.ascontiguousarray(s[b, i0:i0 + ROWS, :].T)
        m["kinT"] = np.ascontiguousarray(k_in[b].T)
        zt = np.ascontiguousarray(z[b, i0:i0 + ROWS].transpose(2, 0, 1))
        zh = zt.astype(ml_dtypes.bfloat16)
        zlo = (zt - zh.astype(np.float32)).astype(ml_dtypes.bfloat16)
        m["zTh"] = zh.view(np.float32)
        m["zTl"] = zlo.view(np.float32)
        if not mask_trivial:
            m["mneg"] = np.ascontiguousarray(
                ((1.0 - mask[b]) * -1e6).reshape(1, N))
        in_maps.append(m)
    return mask_trivial, in_maps


def _run(in_maps, mask_trivial, **kwargs):
    if mask_trivial not in _CACHE:
        _CACHE[mask_trivial] = _build_program(mask_trivial)
    nc = _CACHE[mask_trivial]
    res = run_bass_kernel_spmd(nc, in_maps, core_ids=list(range(N_CORES)),
                               **kwargs)
    out = np.empty((B, N, CS), dtype=np.float32)
    for core in range(N_CORES):
        b, ib = core // 4, core % 4
        out[b, ib * ROWS:(ib + 1) * ROWS, :] = res.results[core]["out"]
    return out, res


def kernel(**inputs):
    mask_trivial, in_maps = _prepare(**inputs)
    out, _ = _run(in_maps, mask_trivial)
    return out


def run_profiled(inputs, tmpdir=None):
    mask_trivial, in_maps = _prepare(**inputs)
    out, res = _run(in_maps, mask_trivial, trace=True, tmpdir=tmpdir)
    return out, res


# revision 14
# speedup vs baseline: 1.1728x; 1.1728x over previous
"""AttentionPairBias kernel for 8 Trainium2 NeuronCores.

Sharding: data-parallel over (batch, query-row-block). Core c handles batch
b = c // 4 and query rows i in [(c % 4) * 128, (c % 4 + 1) * 128).
Each core computes the full 16-head attention for its 128 query rows:
  - q/g projections for its rows; k/v projections for its batch (replicated
    across the 4 cores of the batch).
  - pair bias via the LayerNorm decomposition
      bias[i,j,h] = rsig(i,j) * (zu[i,j,h] - mu(i,j) * su[h]) + t[h]
    with u[:,h] = ln_g * wz[:,h], su = sum_c u, t = ln_b @ wz, so the only
    full-z work is one matmul zT.T @ [u | ones] (done in bf16 on the PE with
    4-way column tiling) plus a squared copy for the variance.
  - z arrives host-transposed as zT [c_z, i, j] so the contraction dim is on
    partitions; it is cast fp32->bf16 during the DMA load.
  - zu / musum / sumsq round-trip through DRAM to switch from [head, (i,j)]
    layout back to [i, j] tiles.
"""

import sys

sys.path.insert(0, "/opt/trn_rl_repo")

from contextlib import ExitStack

import numpy as np

import concourse.bacc as bacc
import concourse.bass as bass
import concourse.mybir as mybir
import concourse.tile as tile
from concourse.bass_utils import run_bass_kernel_spmd
from concourse.masks import make_identity

F32 = mybir.dt.float32
F32R = mybir.dt.float32r
BF16 = mybir.dt.bfloat16
AF = mybir.ActivationFunctionType
ALU = mybir.AluOpType

B, N, CS, CZ, H, D = 2, 512, 1024, 128, 16, 64
ROWS = 128          # query rows per core
NCHUNK = CS // 128  # 8 contraction chunks of 128
N_CORES = 8
EPS = 1e-5

_CACHE = {}


def _build_program(mask_trivial: bool):
    nc = bacc.Bacc("TRN2", target_bir_lowering=False, debug=False,
                   num_devices=N_CORES)

    def din(name, shape):
        return nc.dram_tensor(name, shape, F32, kind="ExternalInput").ap()

    sT_d = din("sT", (CS, ROWS))
    kinT_d = din("kinT", (CS, N))
    # bf16 hi/lo planes of zT, bit-packed into f32-typed tensors (the axon
    # PJRT compile path stalls on non-f32 jit parameters).
    zh_d = din("zTh", (CZ, ROWS, N // 2))
    zl_d = din("zTl", (CZ, ROWS, N // 2))
    wq_d = din("wq", (CS, CS))
    wk_d = din("wk", (CS, CS))
    wv_d = din("wv", (CS, CS))
    wg_d = din("wg", (CS, CS))
    wo_d = din("wo", (CS, CS))
    bq_d = din("bqt", (128, NCHUNK))
    lng_d = din("lng", (CZ, 1))
    lnb_d = din("lnb", (CZ, 1))
    wz_d = din("wz", (CZ, H))
    if not mask_trivial:
        mneg_d = din("mneg", (1, N))
    out_d = nc.dram_tensor("out", (ROWS, CS), F32, kind="ExternalOutput").ap()

    with tile.TileContext(nc) as tc, ExitStack() as ctx:
        dram = ctx.enter_context(tc.tile_pool(name="dram", bufs=1, space="DRAM"))
        zu_d = dram.tile([17, ROWS, N], F32)     # [head|musum, i, j]
        ss_d = dram.tile([ROWS, N], F32)         # sumsq over c per (i, j)

        const = ctx.enter_context(tc.tile_pool(name="const", bufs=1))
        small = ctx.enter_context(tc.tile_pool(name="small", bufs=1))

        ident = const.tile([128, 128], F32)
        make_identity(nc, ident[:])
        ones = const.tile([128, 128], F32)
        nc.vector.memset(ones[:], 1.0)

        wz_sb = small.tile([CZ, H], F32)
        nc.sync.dma_start(wz_sb[:], wz_d[:])
        lng_sb = small.tile([CZ, 1], F32)
        nc.sync.dma_start(lng_sb[:], lng_d[:])
        lnb_sb = small.tile([CZ, 1], F32)
        nc.sync.dma_start(lnb_sb[:], lnb_d[:])
        bq_sb = small.tile([128, NCHUNK], F32)
        nc.sync.dma_start(bq_sb[:], bq_d[:])

        u_f = small.tile([CZ, H], F32)
        nc.vector.tensor_tensor(u_f[:], wz_sb[:],
                                lng_sb[:, 0:1].to_broadcast([CZ, H]), ALU.mult)
        bwz = small.tile([CZ, H], F32)
        nc.vector.tensor_tensor(bwz[:], wz_sb[:],
                                lnb_sb[:, 0:1].to_broadcast([CZ, H]), ALU.mult)
        # stationaries for the z matmul, hi/lo split of u:
        #   u1 = [u_hi (16) | ones | zeros...], u2 = [u_lo (16) | zeros...]
        u_bf = const.tile([CZ, 32], BF16)
        nc.vector.memset(u_bf[:], 0.0)
        nc.vector.tensor_copy(u_bf[:, 0:H], u_f[:])
        nc.vector.memset(u_bf[:, H:H + 1], 1.0)
        u_hi_f = small.tile([CZ, H], F32)
        nc.vector.tensor_copy(u_hi_f[:], u_bf[:, 0:H])
        u_lo = const.tile([CZ, 32], BF16)
        nc.vector.memset(u_lo[:], 0.0)
        u_lo_f = small.tile([CZ, H], F32)
        nc.vector.tensor_tensor(u_lo_f[:], u_f[:], u_hi_f[:], ALU.subtract)
        nc.vector.tensor_copy(u_lo[:, 0:H], u_lo_f[:])

        msu_b = small.tile([128, H], F32)   # -su[h]/128 replicated on partitions
        t_b = small.tile([128, H], F32)
        with ExitStack() as pctx:
            ppre = pctx.enter_context(tc.tile_pool(name="ppre", bufs=1,
                                                   space="PSUM"))
            su_ps = ppre.tile([128, H], F32, tag="pre")
            nc.tensor.matmul(su_ps[:], ones[:], u_f[:], start=True, stop=True)
            nc.vector.tensor_scalar_mul(msu_b[:], su_ps[:], -1.0 / CZ)
            t_ps = ppre.tile([128, H], F32, tag="pre")
            nc.tensor.matmul(t_ps[:], ones[:], bwz[:], start=True, stop=True)
            nc.vector.tensor_copy(t_b[:], t_ps[:])
        bq8 = small.tile([128, NCHUNK], F32)
        nc.vector.tensor_scalar_mul(bq8[:], bq_sb[:], 0.125)

        if not mask_trivial:
            mrow = small.tile([1, N], F32)
            nc.sync.dma_start(mrow[:], mneg_d[:])
            mfull = small.tile([128, N], F32)
            nc.vector.tensor_copy(mfull[:], mrow[0:1, :].to_broadcast([128, N]))

        # ---------------- phase 1: z -> zu / musum / sumsq ----------------
        QR = 4   # query rows per (group, step)
        with ExitStack() as zctx:
            ztp = zctx.enter_context(tc.tile_pool(name="ztp", bufs=8))
            z2p = zctx.enter_context(tc.tile_pool(name="z2p", bufs=8))
            zup = zctx.enter_context(tc.tile_pool(name="zup", bufs=2))
            ssp = zctx.enter_context(tc.tile_pool(name="ssp", bufs=2))
            zps = zctx.enter_context(tc.tile_pool(name="zps", bufs=3, space="PSUM"))

            for o in range(32 // QR):
                zts, zls, z2s = [], [], []
                for g in range(4):
                    r0 = 32 * g + QR * o
                    zt32 = ztp.tile([CZ, QR, N // 2], F32, tag="zt")
                    nc.sync.dma_start(zt32[:], zh_d[:, r0:r0 + QR, :])
                    zl32 = ztp.tile([CZ, QR, N // 2], F32, tag="zl")
                    nc.sync.dma_start(zl32[:], zl_d[:, r0:r0 + QR, :])
                    zt = zt32[:].bitcast(BF16)
                    zl = zl32[:].bitcast(BF16)
                    z2 = z2p.tile([CZ, QR, N], BF16, tag="z2")
                    nc.scalar.activation(z2[:], zt[:], AF.Square)
                    zts.append(zt)
                    zls.append(zl)
                    z2s.append(z2)
                zu_sb = zup.tile([128, QR, N], F32)
                ss_sb = ssp.tile([128, QR, N], F32)
                for kk in range(QR):
                    ps = zps.tile([128, 2 * N], F32)
                    for g in range(4):
                        tp = (0, 32 * g)
                        dst = ps[32 * g:32 * g + 32, 0:N]
                        nc.tensor.matmul(dst, u_bf[:], zts[g][:, kk, :],
                                         start=True, stop=False,
                                         tile_position=tp)
                        nc.tensor.matmul(dst, u_lo[:], zts[g][:, kk, :],
                                         start=False, stop=False,
                                         tile_position=tp)
                        nc.tensor.matmul(dst, u_bf[:], zls[g][:, kk, :],
                                         start=False, stop=True,
                                         tile_position=tp)
                        nc.tensor.matmul(ps[32 * g:32 * g + 32, N:2 * N],
                                         u_bf[:], z2s[g][:, kk, :],
                                         start=True, stop=True,
                                         tile_position=tp)
                    nc.vector.tensor_copy(zu_sb[:, kk, :], ps[:, 0:N])
                    nc.scalar.copy(ss_sb[:, kk, :], ps[:, N:2 * N])
                for g in range(4):
                    r0 = 32 * g + QR * o
                    nc.sync.dma_start(zu_d[:, r0:r0 + QR, :],
                                      zu_sb[32 * g:32 * g + 17, :, :])
                    nc.sync.dma_start(
                        ss_d[r0:r0 + QR, :].rearrange("(o k) j -> o k j", o=1),
                        ss_sb[32 * g + 16:32 * g + 17, :, :])

        # ---------------- phase 2: projections ----------------
        proj = ctx.enter_context(tc.tile_pool(name="proj", bufs=1))
        sTr_sb = proj.tile([128, NCHUNK, ROWS], F32R)
        nc.gpsimd.dma_start(sTr_sb[:], sT_d.rearrange("(cc p) i -> p cc i", p=128))
        kinT_sb = proj.tile([128, NCHUNK, N], F32R)
        nc.gpsimd.dma_start(kinT_sb[:], kinT_d.rearrange("(cc p) j -> p cc j", p=128))

        qT_sb = proj.tile([128, NCHUNK, ROWS], F32)    # (q + bq)/8, [d, i]
        kT_sb = proj.tile([128, NCHUNK, N], F32)       # [d, j]
        v_sb = proj.tile([128, 4, CS], F32)            # [j within chunk, jc, h*64+d]
        g_sb = proj.tile([128, CS], F32)               # sigmoid(s @ wg), [i, c]

        with ExitStack() as wctx:
            wpool = wctx.enter_context(tc.tile_pool(name="wpool", bufs=2))
            prps = wctx.enter_context(tc.tile_pool(name="prps", bufs=2, space="PSUM"))

            wq_sb = wpool.tile([128, NCHUNK, CS], F32R, tag="wr")
            nc.gpsimd.dma_start(wq_sb[:], wq_d.rearrange("(cc p) n -> p cc n", p=128))
            for dc in range(NCHUNK):
                ps = prps.tile([128, ROWS], F32, tag="q")
                for cc in range(NCHUNK):
                    nc.tensor.matmul(ps[:], wq_sb[:, cc, 128 * dc:128 * dc + 128],
                                     sTr_sb[:, cc, :],
                                     start=(cc == 0), stop=(cc == NCHUNK - 1))
                nc.scalar.activation(qT_sb[:, dc, :], ps[:], AF.Identity,
                                     bias=bq8[:, dc:dc + 1], scale=0.125)

            wk_sb = wpool.tile([128, NCHUNK, CS], F32R, tag="wr")
            nc.gpsimd.dma_start(wk_sb[:], wk_d.rearrange("(cc p) n -> p cc n", p=128))
            for dc in range(NCHUNK):
                ps = prps.tile([128, N], F32, tag="k")
                for cc in range(NCHUNK):
                    nc.tensor.matmul(ps[:],
                                     wk_sb[:, cc, 128 * dc:128 * dc + 128],
                                     kinT_sb[:, cc, :],
                                     start=(cc == 0), stop=(cc == NCHUNK - 1))
                nc.scalar.copy(kT_sb[:, dc, :], ps[:])

            wv_sb = wpool.tile([128, NCHUNK, CS], F32R, tag="wr")
            nc.gpsimd.dma_start(wv_sb[:], wv_d.rearrange("(cc p) n -> p cc n", p=128))
            for jc in range(4):
                for nh in range(2):
                    ps = prps.tile([128, 512], F32, tag="v")
                    for cc in range(NCHUNK):
                        nc.tensor.matmul(
                            ps[:],
                            kinT_sb[:, cc, 128 * jc:128 * jc + 128],
                            wv_sb[:, cc, 512 * nh:512 * nh + 512],
                            start=(cc == 0), stop=(cc == NCHUNK - 1))
                    nc.scalar.copy(v_sb[:, jc, 512 * nh:512 * nh + 512], ps[:])

            wg_sb = wpool.tile([128, NCHUNK, CS], F32R, tag="wr")
            nc.gpsimd.dma_start(wg_sb[:], wg_d.rearrange("(cc p) n -> p cc n", p=128))
            for nh in range(2):
                ps = prps.tile([128, 512], F32, tag="v")
                for cc in range(NCHUNK):
                    nc.tensor.matmul(ps[:], sTr_sb[:, cc, :],
                                     wg_sb[:, cc, 512 * nh:512 * nh + 512],
                                     start=(cc == 0), stop=(cc == NCHUNK - 1))
                nc.scalar.activation(g_sb[:, 512 * nh:512 * nh + 512], ps[:],
                                     AF.Sigmoid)

        # ---------------- phase 3: attention ----------------
        att = ctx.enter_context(tc.tile_pool(name="att", bufs=3))
        apool = ctx.enter_context(tc.tile_pool(name="apool", bufs=1))
        spsum = ctx.enter_context(tc.tile_pool(name="spsum", bufs=2, space="PSUM"))
        tpsum = ctx.enter_context(tc.tile_pool(name="tpsum", bufs=2, space="PSUM"))
        opsum = ctx.enter_context(tc.tile_pool(name="opsum", bufs=2, space="PSUM"))

        musum = apool.tile([128, N], F32)
        nc.sync.dma_start(musum[:],
                          zu_d[16:17, :, :].rearrange("o i j -> (o i) j"))
        ssq = apool.tile([128, N], F32)
        nc.sync.dma_start(ssq[:], ss_d[:])
        m2 = apool.tile([128, N], F32)
        nc.vector.tensor_tensor(m2[:], musum[:], musum[:], ALU.mult)
        wvar = apool.tile([128, N], F32)   # 128 * var
        nc.vector.scalar_tensor_tensor(wvar[:], m2[:], -1.0 / CZ, ssq[:],
                                       op0=ALU.mult, op1=ALU.add)
        eps_b = apool.tile([128, 1], F32)
        nc.vector.memset(eps_b[:], EPS)
        sdev = apool.tile([128, N], F32)   # sqrt(var + eps)
        nc.scalar.activation(sdev[:], wvar[:], AF.Sqrt, bias=eps_b[:, 0:1],
                             scale=1.0 / CZ)
        rsig = apool.tile([128, N], F32)
        nc.vector.reciprocal(rsig[:], sdev[:])

        wo_sb = proj.tile([128, NCHUNK, CS], F32R)
        nc.gpsimd.dma_start(wo_sb[:], wo_d.rearrange("(cc p) n -> p cc n", p=128))

        o_all = apool.tile([128, H, D], F32)
        sums = apool.tile([128, H], F32)

        for h in range(H):
            bh = att.tile([128, N], F32, tag="bh")
            nc.vector.tensor_scalar_mul(bh[:], musum[:], msu_b[:, h:h + 1])
            if not mask_trivial:
                nc.vector.tensor_tensor(bh[:], bh[:], mfull[:], ALU.add)
            nc.gpsimd.dma_start(
                bh[:], zu_d[h:h + 1, :, :].rearrange("o i j -> (o i) j"),
                accum_op=ALU.add)
            sc_ps = spsum.tile([128, N], F32, tag="sc")
            p0 = 64 * (h % 2)
            nc.tensor.matmul(sc_ps[:],
                             qT_sb[p0:p0 + 64, h // 2, :],
                             kT_sb[p0:p0 + 64, h // 2, :],
                             start=True, stop=True)
            t2 = att.tile([128, N], F32, tag="t2")
            nc.vector.tensor_tensor(t2[:], bh[:], rsig[:], ALU.mult)
            s_sb = att.tile([128, N], F32, tag="s")
            nc.vector.scalar_tensor_tensor(s_sb[:], t2[:], t_b[:, h:h + 1],
                                           sc_ps[:], op0=ALU.add, op1=ALU.add)
            nm = att.tile([128, 1], F32, tag="nm")
            nc.vector.tensor_reduce(nm[:], s_sb[:], mybir.AxisListType.X,
                                    ALU.max, negate=True)
            p_sb = att.tile([128, N], F32, tag="p")
            nc.scalar.activation(p_sb[:], s_sb[:], AF.Exp, bias=nm[:, 0:1],
                                 accum_out=sums[:, h:h + 1])
            pt_ps = tpsum.tile([128, N], F32, tag="pt")
            for jc in range(4):
                nc.tensor.transpose(pt_ps[:, 128 * jc:128 * jc + 128],
                                    p_sb[:, 128 * jc:128 * jc + 128], ident[:])
            pt_sb = att.tile([128, N], F32, tag="ptsb")
            nc.vector.tensor_copy(pt_sb[:], pt_ps[:])
            o_ps = opsum.tile([128, D], F32, tag="o")
            for jc in range(4):
                nc.tensor.matmul(o_ps[:], pt_sb[:, 128 * jc:128 * jc + 128],
                                 v_sb[:, jc, D * h:D * h + D],
                                 start=(jc == 0), stop=(jc == 3))
            nc.scalar.copy(o_all[:, h, :], o_ps[:])

        recip = apool.tile([128, H], F32)
        nc.vector.reciprocal(recip[:], sums[:])
        go = apool.tile([128, H, D], F32)
        nc.vector.tensor_tensor(go[:], o_all[:],
                                recip[:, :, None].to_broadcast([128, H, D]),
                                ALU.mult)
        gof = go.rearrange("p h d -> p (h d)")
        nc.vector.tensor_tensor(gof[:], gof[:], g_sb[:], ALU.mult)

        goT = apool.tile([128, NCHUNK, ROWS], F32R)
        for ccc in range(NCHUNK):
            gt_ps = tpsum.tile([128, 128], F32, tag="pt")
            nc.tensor.transpose(gt_ps[:], gof[:, 128 * ccc:128 * ccc + 128],
                                ident[:])
            nc.scalar.copy(goT[:, ccc, :], gt_ps[:])

        out_sb = apool.tile([128, CS], F32)
        for nh in range(2):
            ps = spsum.tile([128, 512], F32, tag="sc")
            for cc in range(NCHUNK):
                nc.tensor.matmul(ps[:], goT[:, cc, :],
                                 wo_sb[:, cc, 512 * nh:512 * nh + 512],
                                 start=(cc == 0), stop=(cc == NCHUNK - 1))
            nc.vector.tensor_copy(out_sb[:, 512 * nh:512 * nh + 512], ps[:])
        nc.sync.dma_start(out_d[:], out_sb[:])

    nc.compile()
    return nc


def _prepare(s, z, mask, k_in, wq, bq, wk, wv, wg, ln_g, ln_b, wz, wo,
             multiplicity=1, **_ignored):
    import ml_dtypes
    s = np.asarray(s, dtype=np.float32)
    z = np.asarray(z, dtype=np.float32)
    mask = np.asarray(mask, dtype=np.float32)
    k_in = np.asarray(k_in, dtype=np.float32)
    assert int(multiplicity) == 1, "only multiplicity == 1 is supported"
    mask_trivial = bool(np.all(mask == 1.0))

    shared = {
        "wq": np.ascontiguousarray(wq, dtype=np.float32),
        "wk": np.ascontiguousarray(wk, dtype=np.float32),
        "wv": np.ascontiguousarray(wv, dtype=np.float32),
        "wg": np.ascontiguousarray(wg, dtype=np.float32),
        "wo": np.ascontiguousarray(wo, dtype=np.float32),
        "bqt": np.ascontiguousarray(
            np.asarray(bq, dtype=np.float32).reshape(NCHUNK, 128).T),
        "lng": np.ascontiguousarray(
            np.asarray(ln_g, dtype=np.float32).reshape(CZ, 1)),
        "lnb": np.ascontiguousarray(
            np.asarray(ln_b, dtype=np.float32).reshape(CZ, 1)),
        "wz": np.ascontiguousarray(wz, dtype=np.float32),
    }
    in_maps = []
    for core in range(N_CORES):
        b, ib = core // 4, core % 4
        i0 = ib * ROWS
        m = dict(shared)
        m["sT"] = np.ascontiguousarray(s[b, i0:i0 + ROWS, :].T)
        m["kinT"] = np.ascontiguousarray(k_in[b].T)
        zt = np.ascontiguousarray(z[b, i0:i0 + ROWS].transpose(2, 0, 1))
        zh = zt.astype(ml_dtypes.bfloat16)
        zlo = (zt - zh.astype(np.float32)).astype(ml_dtypes.bfloat16)
        m["zTh"] = zh.view(np.float32)
        m["zTl"] = zlo.view(np.float32)
        if not mask_trivial:
            m["mneg"] = np.ascontiguousarray(
                ((1.0 - mask[b]) * -1e6).reshape(1, N))
        in_maps.append(m)
    return mask_trivial, in_maps


def _run(in_maps, mask_trivial, **kwargs):
    if mask_trivial not in _CACHE:
        _CACHE[mask_trivial] = _build_program(mask_trivial)
    nc = _CACHE[mask_trivial]
    res = run_bass_kernel_spmd(nc, in_maps, core_ids=list(range(N_CORES)),
                               **kwargs)
    out = np.empty((B, N, CS), dtype=np.float32)
    for core in range(N_CORES):
        b, ib = core // 4, core % 4
        out[b, ib * ROWS:(ib + 1) * ROWS, :] = res.results[core]["out"]
    return out, res


def kernel(**inputs):
    mask_trivial, in_maps = _prepare(**inputs)
    out, _ = _run(in_maps, mask_trivial)
    return out


def run_profiled(inputs, tmpdir=None):
    mask_trivial, in_maps = _prepare(**inputs)
    out, res = _run(in_maps, mask_trivial, trace=True, tmpdir=tmpdir)
    return out, res
